# revision 35
# baseline (speedup 1.0000x reference)
"""MoE feed-forward (top-2 of 8 experts) Trainium2 Bass kernel.

Expert-parallel across 8 NeuronCores with sparse top-2 routing:

- Every core computes gating for all 4096 tokens (identical top-2 +
  softmax on each core); a per-core one-hot selects this expert's
  combine weight per token (0 for unrouted tokens).
- Stream compaction: inclusive cumsum over the routing mask via
  triangular-matrix matmuls gives each routed token a slot in [0, C).
- The slot->(token, weight) table is built fully on-chip: per token
  tile a 0/1 selection matrix sel[t, slot] = (slot(t) == slot) is
  formed on the vector engine (fp16 iota compare) and matmul'd against
  [ti, p, w] columns, accumulating metaT [4, C] in PSUM; per-128-slot
  blocks are PE-transposed back to partition-major to produce gather
  indices and combine weights.
- FFN (bf16 matmuls, fp32 accumulate, exact Gelu) runs over the C=1280
  compacted slots: gather bf16 x rows by token id, PE-transpose to
  feature-major, W1+Gelu, W2, scale by combine weight, scatter bf16
  rows back to a zeroed dense [4096+128, 1024] partial buffer.
- ReduceScatter (bf16) sums the 8 expert partials; each core applies
  residual + LayerNorm to its 512-token shard; host concatenates.
"""

from contextlib import ExitStack

import numpy as np
import ml_dtypes

import concourse.bass as bass
import concourse.bacc as bacc
import concourse.tile as tile
from concourse import mybir
from concourse.bass_utils import run_bass_kernel_spmd

FP32 = mybir.dt.float32
FP16 = mybir.dt.float16
BF16 = mybir.dt.bfloat16
INT32 = mybir.dt.int32
AF = mybir.ActivationFunctionType
ALU = mybir.AluOpType

B, T, D, H, E = 2, 2048, 1024, 4096, 8
N = B * T            # 4096 tokens
NCORES = 8
TPC = N // NCORES    # 512 tokens output shard per core
P = 128
KD = D // P          # 8 contraction tiles over D
KH = H // P          # 32 contraction tiles over H
NT = N // P          # 32 token tiles (routing)
C = 1280             # capacity: compacted tokens processed per expert
NB = C // P          # 10 slot blocks
LN_EPS = 1e-5
GROUPS = [(g * 256, 256) for g in range(5)]    # FFN slot groups
CHUNKS = [(0, 512), (512, 512), (1024, 256)]   # metaT col chunks


def _r1(ap):
    return ap.rearrange("p (n o) -> p n o", o=1)


def build_program(debug=False):
    nc = bacc.Bacc("TRN2", target_bir_lowering=False, num_devices=NCORES)
    if debug:
        dbg_wall = nc.dram_tensor("dbg_wall", [P, NT], FP32,
                                  kind="ExternalOutput")
        dbg_of = nc.dram_tensor("dbg_of", [P, NT], FP32,
                                kind="ExternalOutput")
        dbg_xg = nc.dram_tensor("dbg_xg", [C + P, 2], FP32,
                                kind="ExternalOutput")
        dbg_rs = nc.dram_tensor("dbg_rs", [TPC, D], FP32,
                                kind="ExternalOutput")

    xT = nc.dram_tensor("xTt", [N, D], BF16, kind="ExternalInput")
    xrb = nc.dram_tensor("xrb", [N + 1, D], BF16, kind="ExternalInput")
    xs = nc.dram_tensor("xs", [TPC, D], FP32, kind="ExternalInput")
    Wg = nc.dram_tensor("Wg", [D, E], BF16, kind="ExternalInput")
    bg = nc.dram_tensor("bg", [1, E], FP32, kind="ExternalInput")
    W1 = nc.dram_tensor("W1e", [D, H], BF16, kind="ExternalInput")
    b1 = nc.dram_tensor("b1e", [1, H], FP32, kind="ExternalInput")
    W2 = nc.dram_tensor("W2e", [H, D], BF16, kind="ExternalInput")
    b2 = nc.dram_tensor("b2e", [1, D], FP32, kind="ExternalInput")
    eoh = nc.dram_tensor("eoh", [1, E], FP32, kind="ExternalInput")
    gam = nc.dram_tensor("gamma", [1, D], FP32, kind="ExternalInput")
    bet = nc.dram_tensor("beta", [1, D], FP32, kind="ExternalInput")
    tri = nc.dram_tensor("tri", [P, P], FP32, kind="ExternalInput")
    tris = nc.dram_tensor("tris", [NT, NT], FP32, kind="ExternalInput")
    ones1 = nc.dram_tensor("ones1", [1, P], FP32, kind="ExternalInput")
    iotaC = nc.dram_tensor("iotaC", [1, C], FP16, kind="ExternalInput")
    pcol = nc.dram_tensor("pcol", [P, 1], FP32, kind="ExternalInput")
    eye = nc.dram_tensor("eye", [P, P], BF16, kind="ExternalInput")
    zrowb = nc.dram_tensor("zrowb", [1, D], BF16, kind="ExternalInput")
    out = nc.dram_tensor("out", [TPC, D], FP32, kind="ExternalOutput")

    Wg_t = Wg.rearrange("(kd p) e -> p kd e", p=P)
    W1_t = W1.rearrange("(kd p) h -> p kd h", p=P)
    W2_t = W2.rearrange("(hk p) d -> p hk d", p=P)
    b1_t = b1.rearrange("o (hk p) -> p (o hk)", p=P)

    with ExitStack() as ctx:
        tc = ctx.enter_context(tile.TileContext(nc))
        singles = ctx.enter_context(tc.tile_pool(name="singles", bufs=1))
        xf_pool = ctx.enter_context(tc.tile_pool(name="xf", bufs=3))
        rt_pool = ctx.enter_context(tc.tile_pool(name="rt", bufs=1))
        sel_pool = ctx.enter_context(tc.tile_pool(name="sel", bufs=2))
        mrow_pool = ctx.enter_context(tc.tile_pool(name="mrow", bufs=12))
        xt_pool = ctx.enter_context(tc.tile_pool(name="xt", bufs=2))
        xg_pool = ctx.enter_context(tc.tile_pool(name="xg", bufs=1))
        xb_pool = ctx.enter_context(tc.tile_pool(name="xb", bufs=1))
        h_pool = ctx.enter_context(tc.tile_pool(name="h", bufs=1))
        y_pool = ctx.enter_context(tc.tile_pool(name="y", bufs=2))
        ln_pool = ctx.enter_context(tc.tile_pool(name="ln", bufs=2))
        ps_rt = ctx.enter_context(tc.tile_pool(name="ps_rt", bufs=2, space="PSUM"))
        ps_tp = ctx.enter_context(tc.tile_pool(name="ps_tp", bufs=1, space="PSUM"))
        ps_h = ctx.enter_context(tc.tile_pool(name="ps_h", bufs=2, space="PSUM"))
        ps_y = ctx.enter_context(tc.tile_pool(name="ps_y", bufs=1, space="PSUM"))
        dram = ctx.enter_context(tc.tile_pool(name="dram", bufs=1, space="DRAM"))

        partial = dram.tile([N + P, D], BF16)
        rs_out = dram.tile([TPC, D], BF16)

        # ---- routing-phase constants (loaded first) ----------------------
        Wgsb = singles.tile([P, KD, E], BF16)
        nc.sync.dma_start(out=Wgsb[:], in_=Wg_t[:])
        bgsb = singles.tile([P, E], FP32)
        nc.sync.dma_start(out=bgsb[:], in_=bg[:].to_broadcast([P, E]))
        eohsb = singles.tile([P, 1, E], FP32)
        nc.sync.dma_start(out=eohsb[:, 0, :], in_=eoh[:].to_broadcast([P, E]))
        onescol = singles.tile([P, 1], FP32)
        nc.vector.memset(onescol[:], 1.0)
        epssb = singles.tile([P, 1], FP32)
        nc.vector.memset(epssb[:], LN_EPS)

        # ---- phase 1: routing logits (bf16 matmul, fp32 psum) ------------
        lg_all = singles.tile([P, NT, E], FP32)
        for ti in range(NT):
            xf = xf_pool.tile([P, KD, P], BF16, tag="xf")
            nc.sync.dma_start(
                out=xf[:].rearrange("p kd q -> p (kd q)"),
                in_=xT[ti * P:(ti + 1) * P, :])
            lg_ps = ps_rt.tile([P, E], FP32, space="PSUM", tag="s")
            for kd in range(KD):
                nc.tensor.matmul(
                    out=lg_ps[:], lhsT=xf[:, kd, :],
                    rhs=Wgsb[:, kd, :],
                    start=(kd == 0), stop=(kd == KD - 1))
            nc.vector.tensor_add(out=lg_all[:, ti, :], in0=lg_ps[:],
                                 in1=bgsb[:])

        trisb = singles.tile([P, P], FP32)
        nc.scalar.dma_start(out=trisb[:], in_=tri[:])
        trissb = singles.tile([NT, NT], FP32)
        nc.scalar.dma_start(out=trissb[:], in_=tris[:])
        ones1sb = singles.tile([1, P], FP32)
        nc.scalar.dma_start(out=ones1sb[:], in_=ones1[:])
        iotasb = singles.tile([P, C], FP16)
        nc.scalar.dma_start(out=iotasb[:], in_=iotaC[:].to_broadcast([P, C]))
        pcolsb = singles.tile([P, 1], FP32)
        nc.scalar.dma_start(out=pcolsb[:], in_=pcol[:])
        eyesb = singles.tile([P, P], BF16)
        nc.scalar.dma_start(out=eyesb[:], in_=eye[:])
        # ---- phase 1b: batched top-2 + softmax over all 32 tiles ---------
        def bb(big, small):
            return bass.broadcast_tensor_aps(big, small)

        m1 = rt_pool.tile([P, NT], FP32, tag="m1")
        nc.vector.reduce_max(out=m1[:], in_=lg_all[:],
                             axis=mybir.AxisListType.X)
        mask1 = rt_pool.tile([P, NT, E], FP32, tag="mask1")
        a, b_ = bb(lg_all[:], _r1(m1[:]))
        nc.vector.tensor_tensor(out=mask1[:], in0=a, in1=b_, op=ALU.is_equal)
        neg = rt_pool.tile([P, NT, E], FP32, tag="neg")
        nc.scalar.mul(neg[:], mask1[:], -1e30)
        nc.vector.tensor_add(out=neg[:], in0=lg_all[:], in1=neg[:])
        m2 = rt_pool.tile([P, NT], FP32, tag="m2")
        nc.vector.reduce_max(out=m2[:], in_=neg[:],
                             axis=mybir.AxisListType.X)
        mask2 = rt_pool.tile([P, NT, E], FP32, tag="mask2")
        a, b_ = bb(neg[:], _r1(m2[:]))
        nc.vector.tensor_tensor(out=mask2[:], in0=a, in1=b_, op=ALU.is_equal)
        # softmax over the two selected logits:
        # s1 = 1/(1+exp(m2-m1)), s2 = exp(m2-m1) * s1
        ex = rt_pool.tile([P, NT], FP32, tag="ex")
        nc.vector.tensor_tensor(out=ex[:], in0=m2[:], in1=m1[:],
                                op=ALU.subtract)
        nc.scalar.activation(out=ex[:], in_=ex[:], func=AF.Exp)
        s1 = rt_pool.tile([P, NT], FP32, tag="s1")
        nc.scalar.add(s1[:], ex[:], 1.0)
        nc.vector.reciprocal(out=s1[:], in_=s1[:])
        s2 = rt_pool.tile([P, NT], FP32, tag="s2")
        nc.vector.tensor_tensor(out=s2[:], in0=ex[:], in1=s1[:],
                                op=ALU.mult)
        wc = rt_pool.tile([P, NT, E], FP32, tag="wc")
        a, b_ = bb(mask1[:], _r1(s1[:]))
        nc.vector.tensor_tensor(out=wc[:], in0=a, in1=b_, op=ALU.mult)
        a, b_ = bb(mask2[:], _r1(s2[:]))
        nc.vector.tensor_tensor(out=mask2[:], in0=a, in1=b_, op=ALU.mult)
        nc.vector.tensor_add(out=wc[:], in0=wc[:], in1=mask2[:])
        a, b_ = bb(wc[:], eohsb[:])
        nc.vector.tensor_tensor(out=wc[:], in0=a, in1=b_, op=ALU.mult)
        wall = rt_pool.tile([P, NT], FP32, tag="wall")
        nc.vector.reduce_sum(out=wall[:], in_=wc[:],
                             axis=mybir.AxisListType.X)

        # ---- phase 1c: compaction offsets via cumsum ---------------------
        maskm = rt_pool.tile([P, NT], FP32, tag="maskm")
        nc.vector.tensor_scalar(out=maskm[:], in0=wall[:], scalar1=0.0,
                                scalar2=None, op0=ALU.is_gt)
        cums_ps = ps_rt.tile([P, NT], FP32, space="PSUM", tag="s")
        nc.tensor.matmul(out=cums_ps[:], lhsT=trisb[:], rhs=maskm[:],
                         start=True, stop=True)
        cums = rt_pool.tile([P, NT], FP32, tag="cums")
        nc.vector.tensor_copy(out=cums[:], in_=cums_ps[:])
        tot_ps = ps_rt.tile([NT, 1], FP32, space="PSUM", tag="s")
        nc.tensor.matmul(out=tot_ps[:], lhsT=maskm[:], rhs=onescol[:],
                         start=True, stop=True)
        totT = rt_pool.tile([NT, 1], FP32, tag="totT")
        nc.vector.tensor_copy(out=totT[:], in_=tot_ps[:])
        pref_ps = ps_rt.tile([NT, 1], FP32, space="PSUM", tag="s")
        nc.tensor.matmul(out=pref_ps[:], lhsT=trissb[:], rhs=totT[:],
                         start=True, stop=True)
        prefT = rt_pool.tile([NT, 1], FP32, tag="prefT")
        nc.vector.tensor_copy(out=prefT[:], in_=pref_ps[:])
        eye32 = rt_pool.tile([NT, NT], FP32, tag="eye32")
        nc.vector.tensor_tensor(out=eye32[:], in0=trisb[0:NT, 0:NT],
                                in1=trissb[:], op=ALU.subtract)
        prefrow_ps = ps_rt.tile([1, NT], FP32, space="PSUM", tag="s")
        nc.tensor.matmul(out=prefrow_ps[:], lhsT=prefT[:],
                         rhs=eye32[:], start=True, stop=True)
        prefrow = rt_pool.tile([1, NT], FP32, tag="prefrow")
        nc.vector.tensor_copy(out=prefrow[:], in_=prefrow_ps[:])
        prefb_ps = ps_rt.tile([P, NT], FP32, space="PSUM", tag="s")
        nc.tensor.matmul(out=prefb_ps[:], lhsT=ones1sb[:], rhs=prefrow[:],
                         start=True, stop=True)
        pos = rt_pool.tile([P, NT], FP32, tag="pos")
        nc.vector.tensor_add(out=pos[:], in0=cums[:], in1=prefb_ps[:])
        # offsets: routed -> min(pos-1, C) ; unrouted -> C
        of32 = rt_pool.tile([P, NT], FP32, tag="of32")
        nc.vector.tensor_scalar(out=of32[:], in0=pos[:], scalar1=1.0,
                                scalar2=float(C), op0=ALU.subtract, op1=ALU.min)
        nc.vector.tensor_tensor(out=of32[:], in0=of32[:], in1=maskm[:],
                                op=ALU.mult)
        onem = rt_pool.tile([P, NT], FP32, tag="onem")
        nc.vector.tensor_scalar(out=onem[:], in0=maskm[:], scalar1=1.0,
                                scalar2=-float(C), op0=ALU.subtract,
                                op1=ALU.mult)
        nc.vector.tensor_add(out=of32[:], in0=of32[:], in1=onem[:])


        # ---- phase 2: on-chip slot->(tile, p, w) table via sel matmuls ---
        # metaT[0, c] = tile index of slot c; [1, c] = partition; [2, c] = w
        metaT_ps = ps_y.tile([4, C], FP32, space="PSUM", tag="y_ps")
        for ti in range(NT):
            st3 = rt_pool.tile([P, 4], BF16, tag="st3")
            nc.vector.memset(st3[:, 0:1], float(ti))
            nc.vector.tensor_copy(out=st3[:, 1:2], in_=pcolsb[:])
            nc.vector.tensor_copy(out=st3[:, 2:3], in_=wall[:, ti:ti + 1])
            nc.vector.memset(st3[:, 3:4], 0.0)
            sel = sel_pool.tile([P, C], BF16, tag="sel")
            bound = min(C, (ti + 1) * P)   # slot(t) <= t
            nc.vector.tensor_scalar(out=sel[:, 0:bound], in0=iotasb[:, 0:bound],
                                    scalar1=of32[:, ti:ti + 1],
                                    scalar2=None, op0=ALU.is_equal)
            for c0, cw in CHUNKS:
                if c0 >= bound:
                    continue
                cwe = min(cw, bound - c0)
                nc.tensor.matmul(out=metaT_ps[:, c0:c0 + cwe],
                                 lhsT=st3[:], rhs=sel[:, c0:c0 + cwe],
                                 start=(ti == c0 // P), stop=(ti == NT - 1))
        metaT = rt_pool.tile([4, C], BF16, tag="metaT")
        nc.vector.tensor_copy(out=metaT[:], in_=metaT_ps[:])

        # per-block transpose back to partition-major [128, 4]
        wcols = []
        oys = []
        for j in range(NB):
            tp4 = ps_tp.tile([P, P], BF16, space="PSUM", tag="tp")
            nc.tensor.transpose(out=tp4[0:P, 0:4],
                                in_=metaT[:, j * P:(j + 1) * P],
                                identity=eyesb[0:4, 0:4])
            m4 = mrow_pool.tile([P, 4], FP32, tag="m4")
            nc.vector.tensor_copy(out=m4[:], in_=tp4[0:P, 0:4])
            oyf = mrow_pool.tile([P, 1], FP32, tag="oyf")
            nc.vector.tensor_scalar(out=oyf[:], in0=m4[:, 0:1],
                                    scalar1=float(P), scalar2=None,
                                    op0=ALU.mult)
            nc.vector.tensor_add(out=oyf[:], in0=oyf[:], in1=m4[:, 1:2])
            oy = mrow_pool.tile([P, 1], INT32, tag="oy")
            nc.vector.tensor_copy(out=oy[:], in_=oyf[:])
            wcol = mrow_pool.tile([P, 1], FP32, tag="wcol")
            nc.vector.tensor_copy(out=wcol[:], in_=m4[:, 2:3])
            wcols.append(wcol)
            oys.append(oy)

        # pad slots (never selected) produce metaT columns of all-zero:
        # token id 0 with weight 0 -> harmless: gathers row 0, scales by 0,
        # scatters zeros to partial row 0... BUT partial row 0 is a real row.
        # Guard: w==0 rows must scatter to the dump row N instead.
        # oy' = oy + (w <= 0) * N  (w > 0 for every real slot)
        for j in range(NB):
            gz = mrow_pool.tile([P, 1], FP32, tag="gz")
            nc.vector.tensor_scalar(out=gz[:], in0=wcols[j][:], scalar1=0.0,
                                    scalar2=float(N), op0=ALU.is_le,
                                    op1=ALU.mult)
            gzi = mrow_pool.tile([P, 1], INT32, tag="gzi")
            nc.vector.tensor_copy(out=gzi[:], in_=gz[:])
            nc.vector.tensor_tensor(out=oys[j][:], in0=oys[j][:],
                                    in1=gzi[:], op=ALU.add)

        if debug:
            nc.sync.dma_start(out=dbg_wall[:], in_=wall[:])
            nc.sync.dma_start(out=dbg_of[:], in_=of32[:])
            for j in range(NB):
                dpair = mrow_pool.tile([P, 2], FP32, tag="dpair")
                nc.vector.tensor_copy(out=dpair[:, 0:1], in_=oys[j][:])
                nc.vector.tensor_copy(out=dpair[:, 1:2], in_=wcols[j][:])
                nc.sync.dma_start(out=dbg_xg[j * P:(j + 1) * P, :],
                                  in_=dpair[:])

        # ---- FFN-phase constants + zero-fill (after routing in program
        # order so routing DMAs get priority) ------------------------------
        W1sb = singles.tile([P, KD, H], BF16)
        nc.scalar.dma_start(out=W1sb[:], in_=W1_t[:])
        W2sb = singles.tile([P, KH, D], BF16)
        nc.scalar.dma_start(out=W2sb[:], in_=W2_t[:])
        b1sb = singles.tile([P, KH], FP32)
        nc.scalar.dma_start(out=b1sb[:], in_=b1_t[:])
        b2sb = singles.tile([P, D], FP32)
        nc.scalar.dma_start(out=b2sb[:], in_=b2[:].to_broadcast([P, D]))
        for k in range(N // P + 1):
            nc.scalar.dma_start(out=partial[k * P:(k + 1) * P, :],
                              in_=zrowb[:].to_broadcast([P, D]))

        # ---- phase 3: FFN over compacted slots ---------------------------
        # all gathers issued up front so they are not stuck behind the
        # y-scatters in the single qPoolDynamic FIFO
        xgts = []
        for j in range(NB):
            xgt = xg_pool.tile([P, D], BF16, tag="xg" + str(j % 2))
            nc.gpsimd.indirect_dma_start(
                out=xgt[:], out_offset=None,
                in_=xrb[:], in_offset=bass.IndirectOffsetOnAxis(
                    ap=oys[j][:, 0:1], axis=0))
            xgts.append(xgt)
        for g0, G in GROUPS:
            nts = G // P
            xbT = xb_pool.tile([P, KD, 256], BF16, tag="xbT")
            for ts in range(nts):
                j = (g0 + ts * P) // P
                xgt = xgts[j]
                for kd in range(KD):
                    tps = ps_tp.tile([P, P], BF16, space="PSUM", tag="tp")
                    nc.tensor.transpose(out=tps[:],
                                        in_=xgt[:, kd * P:(kd + 1) * P],
                                        identity=eyesb[:])
                    nc.vector.tensor_copy(
                        out=xbT[:, kd, ts * P:(ts + 1) * P], in_=tps[:])
            hT = h_pool.tile([P, KH, 256], BF16, tag="hT")
            for hk in range(KH):
                h_ps = ps_h.tile([P, G], FP32, space="PSUM")
                for kd in range(KD):
                    nc.tensor.matmul(
                        out=h_ps[:], lhsT=W1sb[:, kd, hk * P:(hk + 1) * P],
                        rhs=xbT[:, kd, 0:G],
                        start=(kd == 0), stop=(kd == KD - 1))
                nc.scalar.activation(
                    out=hT[:, hk, 0:G], in_=h_ps[:], func=AF.Gelu,
                    bias=b1sb[:, hk:hk + 1], scale=1.0)
            for ts in range(nts):
                j = (g0 + ts * P) // P
                y_ps = ps_y.tile([P, D], FP32, space="PSUM", tag="y_ps")
                for hk in range(KH):
                    lhsT = hT[:, hk, ts * P:(ts + 1) * P]
                    for dh in range(2):
                        nc.tensor.matmul(
                            out=y_ps[:, dh * 512:(dh + 1) * 512],
                            lhsT=lhsT,
                            rhs=W2sb[:, hk, dh * 512:(dh + 1) * 512],
                            start=(hk == 0), stop=(hk == KH - 1))
                y_sb = y_pool.tile([P, D], FP32, tag="y")
                nc.vector.tensor_add(out=y_sb[:], in0=y_ps[:], in1=b2sb[:])
                y_bf = y_pool.tile([P, D], BF16, tag="ybf")
                nc.vector.tensor_scalar_mul(out=y_bf[:], in0=y_sb[:],
                                            scalar1=wcols[j][:])
                nc.gpsimd.indirect_dma_start(
                    out=partial[:], out_offset=bass.IndirectOffsetOnAxis(
                        ap=oys[j][:], axis=0),
                    in_=y_bf[:], in_offset=None)

        # ---- phase 4: ReduceScatter + residual + LayerNorm ---------------
        nc.gpsimd.collective_compute(
            "ReduceScatter", ALU.add,
            replica_groups=[list(range(NCORES))],
            ins=[partial[0:N, :].opt()], outs=[rs_out.opt()])

        if debug:
            for ti in range(TPC // P):
                drs = xt_pool.tile([P, D], BF16, tag="xt")
                nc.sync.dma_start(out=drs[:],
                                  in_=rs_out[ti * P:(ti + 1) * P, :])
                drs32 = y_pool.tile([P, D], FP32, tag="y")
                nc.vector.tensor_copy(out=drs32[:], in_=drs[:])
                nc.sync.dma_start(out=dbg_rs[ti * P:(ti + 1) * P, :],
                                  in_=drs32[:])

        gamsb = h_pool.tile([P, D], FP32, tag="hT")
        nc.sync.dma_start(out=gamsb[:], in_=gam[:].to_broadcast([P, D]))
        betsb = xb_pool.tile([P, D], FP32, tag="xbT")
        nc.sync.dma_start(out=betsb[:], in_=bet[:].to_broadcast([P, D]))
        for ti in range(TPC // P):
            rb = xt_pool.tile([P, D], BF16, tag="xt")
            nc.sync.dma_start(out=rb[:], in_=rs_out[ti * P:(ti + 1) * P, :])
            r = y_pool.tile([P, D], FP32, tag="y")
            nc.vector.tensor_copy(out=r[:], in_=rb[:])
            xres = y_pool.tile([P, D], FP32, tag="ybf2")
            nc.sync.dma_start(out=xres[:], in_=xs[ti * P:(ti + 1) * P, :])
            nc.vector.tensor_add(out=r[:], in0=r[:], in1=xres[:])
            stats = ln_pool.tile([P, 2, 6], FP32, tag="stats")
            rr = r[:].rearrange("p (s f) -> p s f", s=2)
            for s in range(2):
                nc.vector.bn_stats(out=stats[:, s, :], in_=rr[:, s, :])
            mv = ln_pool.tile([P, 2], FP32, tag="mv")
            nc.vector.bn_aggr(out=mv[:], in_=stats[:])
            rstd = ln_pool.tile([P, 1], FP32, tag="rstd")
            nc.scalar.activation(out=rstd[:], in_=mv[:, 1:2], func=AF.Sqrt,
                                 bias=epssb[:], scale=1.0)
            nc.vector.reciprocal(out=rstd[:], in_=rstd[:])
            nc.vector.tensor_scalar(
                out=r[:], in0=r[:], scalar1=mv[:, 0:1], scalar2=rstd[:],
                op0=ALU.subtract, op1=ALU.mult)
            nc.vector.tensor_tensor(out=r[:], in0=r[:], in1=gamsb[:],
                                    op=ALU.mult)
            nc.vector.tensor_add(out=r[:], in0=r[:], in1=betsb[:])
            nc.sync.dma_start(out=out[ti * P:(ti + 1) * P, :], in_=r[:])

    nc.compile()
    return nc


_NC_CACHE = None


def _get_program():
    global _NC_CACHE
    if _NC_CACHE is None:
        _NC_CACHE = build_program()
    return _NC_CACHE


def make_in_maps(x, Wg, bg, W1, b1, W2, b2, gamma, beta):
    xf = np.ascontiguousarray(x.reshape(N, D).astype(np.float32))
    xb = xf.astype(ml_dtypes.bfloat16)
    # xTt[ti*P + p, kd*P + q] = x[ti*P + q, kd*P + p] so that one routing
    # tile load is a fully contiguous [P, D] DMA
    xTt = np.ascontiguousarray(
        xb.reshape(NT, P, KD, P).transpose(0, 3, 2, 1).reshape(N, D))
    xrb = np.zeros((N + 1, D), ml_dtypes.bfloat16)
    xrb[:N] = xb
    Wg2 = np.ascontiguousarray(Wg.astype(ml_dtypes.bfloat16))
    bg2 = np.ascontiguousarray(bg.astype(np.float32).reshape(1, E))
    gam = np.ascontiguousarray(gamma.astype(np.float32).reshape(1, D))
    bet = np.ascontiguousarray(beta.astype(np.float32).reshape(1, D))
    tri = np.triu(np.ones((P, P), np.float32))
    tris = np.triu(np.ones((NT, NT), np.float32), k=1)
    ones1 = np.ones((1, P), np.float32)
    iotaC = np.arange(C, dtype=np.float16).reshape(1, C)
    pcol = np.arange(P, dtype=np.float32).reshape(P, 1)
    zrowb = np.zeros((1, D), ml_dtypes.bfloat16)
    in_maps = []
    for e in range(NCORES):
        onehot = np.zeros((1, E), np.float32)
        onehot[0, e] = 1.0
        in_maps.append({
            "xTt": xTt,
            "xrb": xrb,
            "xs": np.ascontiguousarray(xf[e * TPC:(e + 1) * TPC]),
            "Wg": Wg2,
            "bg": bg2,
            "W1e": np.ascontiguousarray(W1[e].astype(ml_dtypes.bfloat16)),
            "b1e": np.ascontiguousarray(b1[e].astype(np.float32).reshape(1, H)),
            "W2e": np.ascontiguousarray(W2[e].astype(ml_dtypes.bfloat16)),
            "b2e": np.ascontiguousarray(b2[e].astype(np.float32).reshape(1, D)),
            "eoh": onehot,
            "gamma": gam,
            "beta": bet,
            "tri": tri,
            "tris": tris,
            "ones1": ones1,
            "iotaC": iotaC,
            "pcol": pcol,
            "eye": np.eye(P).astype(ml_dtypes.bfloat16),
            "zrowb": zrowb,
        })
    return in_maps


def kernel(x, Wg, bg, W1, b1, W2, b2, gamma, beta, _trace=False):
    nc = _get_program()
    in_maps = make_in_maps(x, Wg, bg, W1, b1, W2, b2, gamma, beta)
    res = run_bass_kernel_spmd(
        nc, in_maps, core_ids=list(range(NCORES)), trace=_trace)
    outs = [res.results[c]["out"] for c in range(NCORES)]
    full = np.concatenate(outs, axis=0).reshape(B, T, D).astype(np.float32)
    if _trace:
        kernel.last_results = res
    return full
